# revision 12
# baseline (speedup 1.0000x reference)
"""DenseGCN (3x EdgeConv max-aggr) on 8 TRN2 NeuronCores.

Key algebra: EdgeConv message relu([x_d, x_s - x_d] @ W + b) with W=[Wt;Wb]
= relu(x_d@(Wt-Wb) + x_s@Wb + b). relu and the per-dst add are monotone, so
segment_max commutes: agg[v] = relu(A[v] + b + max_{e:dst=v} B[src_e]) with
A = x@Wt - x@Wb, B = x@Wb -- node-level matmuls only, no per-edge matmul.

Sharding: dst-partition nodes (2500/core, padded to 2560 slots, sorted by
in-degree desc). Per block: local matmuls (PE), AllGather of the B table
(f32 [8*2560+1, 64] with a -1e30 dummy row), then per-round 128-row
indirect-DMA gathers + DVE tensor max. Final quad max-pool on DVE.
"""
import sys, os, types

sys.path.insert(0, "/opt/trn_rl_repo")

import numpy as np


def _setup_trace_hook():
    """Register the NTFF profile hook (missing antenv.axon_hooks shim) so
    run_bass_kernel_spmd(trace=True) works. Safe no-op on failure."""
    try:
        import antenv

        if "antenv.axon_hooks" not in sys.modules:
            m = types.ModuleType("antenv.axon_hooks")
            hook = [None]
            m.set_axon_ntff_profile_hook = lambda h: hook.__setitem__(0, h)
            m.get_axon_ntff_profile_hook = lambda: hook[0]
            sys.modules["antenv.axon_hooks"] = m
            antenv.axon_hooks = m
            from trn_agent_boot.trn_boot import _ntff_profile_via_ctypes

            m.set_axon_ntff_profile_hook(
                _ntff_profile_via_ctypes("/opt/axon/libaxon_pjrt.so")
            )
        import concourse.bass_utils as bu

        bu.upload_artifacts = lambda tmpdir: tmpdir
        return True
    except Exception:
        return False

N_NODES = 20000
N_EDGES = 320000
C_IN = 64
GROWTH = 64
N_BLOCKS = 3
N_CORES = 8
NL = N_NODES // N_CORES          # 2500 local nodes
SLOTS = 2560                     # padded to 20*128
JBLK = SLOTS // 128              # 20 slot-blocks
TBL_ROWS = N_CORES * SLOTS + 1   # 20481, last row = -1e30 dummy
DUMMY = N_CORES * SLOTS


def _build_call_grid(edge_index):
    """Host-side (pure index manipulation): degree-sort nodes per core,
    build the per-round gather call grid, identical structure across cores."""
    src, dst = edge_index[0].astype(np.int64), edge_index[1].astype(np.int64)
    core_of = dst // NL
    perms = []          # per core: slot -> node (global id), len SLOTS (pad: -1)
    slot_of = np.full(N_NODES, -1, np.int64)
    edges_by_core = []
    for m in range(N_CORES):
        lo, hi = m * NL, (m + 1) * NL
        deg = np.bincount(dst[(dst >= lo) & (dst < hi)] - lo, minlength=NL)
        order = np.argsort(-deg, kind="stable")      # local node order by deg desc
        nodes = order + lo
        perm = np.concatenate([nodes, np.full(SLOTS - NL, -1, np.int64)])
        perms.append(perm)
        slot_of[nodes] = np.arange(NL)
        edges_by_core.append((deg, order))
    # global table row of node n: core(n)*SLOTS + local slot
    grow = (np.arange(N_NODES) // NL) * SLOTS + slot_of
    # per core, per slot: list of src rows (table row ids)
    per_core_srcs = []
    maxdeg_per_round = None
    for m in range(N_CORES):
        lo = m * NL
        mask = core_of == m
        d_loc = dst[mask] - lo
        s_rows = grow[src[mask]]
        sl = slot_of[dst[mask]]
        o = np.argsort(sl, kind="stable")
        sl, s_rows = sl[o], s_rows[o]
        starts = np.searchsorted(sl, np.arange(NL))
        ends = np.searchsorted(sl, np.arange(NL) + 1)
        per_core_srcs.append((starts, ends, s_rows))
    degs = np.stack([
        per_core_srcs[m][1] - per_core_srcs[m][0] for m in range(N_CORES)
    ])  # [8, NL] in slot order (desc within each core)
    R = int(degs.max())
    # n_r[r] = max over cores of count(slots with deg > r), rounded up to 128
    n_r = [SLOTS]  # round 0 covers all slots (direct write incl. dummy -1e30)
    for r in range(1, R):
        c = int((degs > r).sum(axis=1).max())
        n_r.append(((c + 127) // 128) * 128)
    calls = []  # list of (round, slot_block)
    for r, n in enumerate(n_r):
        for c in range(n // 128):
            calls.append((r, c))
    NCALLS = len(calls)
    idx_all = np.full((N_CORES, 128, NCALLS), DUMMY, np.int32)
    for m in range(N_CORES):
        starts, ends, s_rows = per_core_srcs[m]
        for k, (r, c) in enumerate(calls):
            base = c * 128
            for p in range(128):
                s = base + p
                if s < NL and starts[s] + r < ends[s]:
                    idx_all[m, p, k] = s_rows[starts[s] + r]
    return perms, calls, idx_all


def kernel(x, W0, b0, W1, b1, W2, b2, edge_index):
    import concourse.bacc as bacc
    import concourse.bass as bass
    import concourse.mybir as mybir
    import concourse.tile as tile
    from concourse.tile import add_dep_helper
    from concourse.masks import make_identity
    from concourse.bass_utils import run_bass_kernel_spmd

    x = np.asarray(x, np.float32)
    Ws = [np.asarray(W0, np.float32), np.asarray(W1, np.float32), np.asarray(W2, np.float32)]
    bs = [np.asarray(b0, np.float32), np.asarray(b1, np.float32), np.asarray(b2, np.float32)]
    edge_index = np.asarray(edge_index)

    perms, calls, idx_all = _build_call_grid(edge_index)
    NCALLS = len(calls)
    CS = [C_IN + k * GROWTH for k in range(N_BLOCKS)]   # 64,128,192
    WT_TILES = [1, 1, 2]  # number of 128-row sbuf tiles per Wt/Wb handled via slicing

    f32 = mybir.dt.float32
    nc = bacc.Bacc("TRN2", target_bir_lowering=False, debug=False, num_devices=N_CORES)

    XT0in = nc.declare_dram_parameter("XT0in", [64, SLOTS], f32, isOutput=False)
    Xnm = nc.declare_dram_parameter("Xnm", [128, JBLK * 64], f32, isOutput=False)
    Wp = [nc.declare_dram_parameter(f"W{k}", [2 * CS[k], 64], f32, isOutput=False) for k in range(3)]
    Bb = [nc.declare_dram_parameter(f"bb{k}", [128, JBLK * 64], f32, isOutput=False) for k in range(3)]
    IdxP = nc.declare_dram_parameter("IdxP", [128, NCALLS], mybir.dt.int32, isOutput=False)
    OutP = nc.declare_dram_parameter("Out", [SLOTS, 64], f32, isOutput=True)

    Btab = nc.dram_tensor("btab", [TBL_ROWS, 64], f32)
    Bloc = nc.dram_tensor("bloc", [SLOTS, 64], f32)

    with tile.TileContext(nc) as tc:
        with (
            tc.tile_pool(name="big", bufs=1) as bigp,
            tc.tile_pool(name="g", bufs=8) as gp,
            tc.tile_pool(name="ps", bufs=4, space="PSUM") as psp,
        ):
            xT0 = bigp.tile([128, SLOTS], f32, tag="xT0")
            xT1 = bigp.tile([128, SLOTS], f32, tag="xT1")
            Xcat = bigp.tile([128, JBLK, 256], f32, tag="Xcat")
            M = bigp.tile([128, JBLK, 64], f32, tag="M")
            As = bigp.tile([128, JBLK, 64], f32, tag="As")
            Bs = bigp.tile([128, JBLK, 64], f32, tag="Bs")
            Idx = bigp.tile([128, NCALLS], mybir.dt.int32, tag="Idx")
            # W chunk layout: each (block, part, chunk) gets its own 64-col slot, rows [0:K)
            wchunks = {}  # (k, part) -> list of (slot, row_in_W, K, x_chan_offset)
            slot = 0
            for k in range(N_BLOCKS):
                C = CS[k]
                for part in (0, 1):
                    lst = []
                    r = 0
                    while r < C:
                        kk = min(128, C - r)
                        lst.append((slot, part * C + r, kk, r))
                        slot += 1
                        r += kk
                    wchunks[(k, part)] = lst
            NSLOTS = slot
            Wt = bigp.tile([128, NSLOTS * 64], f32, tag="Wt")
            bt = bigp.tile([128, JBLK, 64], f32, tag="bt")
            ident = bigp.tile([128, 128], f32, tag="ident")
            neg = bigp.tile([128, 64], f32, tag="neg")

            make_identity(nc, ident[:])
            ld = []
            ld.append(nc.sync.dma_start(out=xT0[0:64, :], in_=XT0in[:]))
            ld.append(nc.sync.dma_start(out=Xcat[:, :, 0:64], in_=Xnm[:].rearrange("p (j c) -> p j c", c=64)))
            ld.append(nc.sync.dma_start(out=Idx[:], in_=IdxP[:]))
            for k in range(3):
                for part in (0, 1):
                    for (sl, wrow, kk, xoff) in wchunks[(k, part)]:
                        ld.append(nc.sync.dma_start(
                            out=Wt[0:kk, sl * 64 : (sl + 1) * 64],
                            in_=Wp[k][wrow : wrow + kk, :],
                        ))
            mneg = nc.gpsimd.memset(neg[:], -1e30)
            dummy_w = nc.sync.dma_start(out=Btab[DUMMY : DUMMY + 1, :], in_=neg[0:1, :])
            add_dep_helper(dummy_w.ins, mneg.ins, sync=True, reason="dummy row after memset")

            prev = dummy_w
            for k in range(N_BLOCKS):
                C = CS[k]
                bldA = nc.sync.dma_start(out=bt[:].rearrange("p j c -> p (j c)"), in_=Bb[k][:])
                add_dep_helper(bldA.ins, prev.ins, sync=True, reason="serialize blocks")

                # 1) matmuls: P (x@Wt) and B (x@Wb) per 128-node tile
                mm_last = None
                for t in range(JBLK):
                    for part in (0, 1):
                        ps = psp.tile([128, 64], f32, tag="mmps")
                        chunks = wchunks[(k, part)]
                        for ci, (sl, wrow, kk, xoff) in enumerate(chunks):
                            xt_tile = xT0 if xoff < 128 else xT1
                            xo = xoff if xoff < 128 else xoff - 128
                            mm = nc.tensor.matmul(
                                out=ps[:, :],
                                lhsT=xt_tile[xo : xo + kk, t * 128 : (t + 1) * 128],
                                rhs=Wt[0:kk, sl * 64 : (sl + 1) * 64],
                                start=(ci == 0),
                                stop=(ci == len(chunks) - 1),
                            )
                            if k > 0:
                                add_dep_helper(mm.ins, prev.ins, sync=True, reason="x ready")
                        tgt = As if part == 0 else Bs
                        cp = nc.vector.tensor_copy(out=tgt[:, t, :], in_=ps[:, :])
                        mm_last = cp
                # A = P - B
                sub = nc.vector.tensor_tensor(out=As[:].rearrange("p j c -> p (j c)"),
                                              in0=As[:].rearrange("p j c -> p (j c)"),
                                              in1=Bs[:].rearrange("p j c -> p (j c)"),
                                              op=mybir.AluOpType.subtract)
                # 2) write local B slab -> DRAM, AllGather into table
                bw = nc.sync.dma_start(
                    out=Bloc[:].rearrange("(j p) c -> p j c", p=128), in_=Bs[:]
                )
                add_dep_helper(bw.ins, mm_last.ins, sync=True, reason="B ready")
                ag = nc.gpsimd.collective_compute(
                    "AllGather", mybir.AluOpType.bypass,
                    replica_groups=[list(range(N_CORES))],
                    ins=[Bloc[:]],
                    outs=[Btab[0 : N_CORES * SLOTS, :]],
                )
                add_dep_helper(ag.ins, bw.ins, sync=True, reason="allgather after write")
                add_dep_helper(ag.ins, prev.ins, sync=True, reason="WAR: prior block gathers done")

                # 3) gather + max rounds
                last_mx = {}
                for kcall, (r, c) in enumerate(calls):
                    gt = gp.tile([128, 64], f32, tag="gt")
                    if r == 0:
                        gcall = nc.gpsimd.indirect_dma_start(
                            out=M[:, c, :], out_offset=None, in_=Btab[:],
                            in_offset=bass.IndirectOffsetOnAxis(ap=Idx[:, kcall : kcall + 1], axis=0),
                        )
                        add_dep_helper(gcall.ins, ag.ins, sync=True, reason="table ready")
                        last_mx[c] = gcall
                    else:
                        gcall = nc.gpsimd.indirect_dma_start(
                            out=gt[:, :], out_offset=None, in_=Btab[:],
                            in_offset=bass.IndirectOffsetOnAxis(ap=Idx[:, kcall : kcall + 1], axis=0),
                        )
                        add_dep_helper(gcall.ins, ag.ins, sync=True, reason="table ready")
                        mx = nc.vector.tensor_tensor(
                            out=M[:, c, :], in0=M[:, c, :], in1=gt[:, :],
                            op=mybir.AluOpType.max,
                        )
                        add_dep_helper(mx.ins, gcall.ins, sync=True, reason="gather done")
                        last_mx[c] = mx

                # 4) epilogue: agg = relu(A + b + M) -> Xcat slice
                a1 = nc.vector.tensor_tensor(
                    out=M[:].rearrange("p j c -> p (j c)"),
                    in0=M[:].rearrange("p j c -> p (j c)"),
                    in1=As[:].rearrange("p j c -> p (j c)"),
                    op=mybir.AluOpType.add,
                )
                add_dep_helper(a1.ins, sub.ins, sync=True, reason="A ready")
                for c in last_mx:
                    add_dep_helper(a1.ins, last_mx[c].ins, sync=True, reason="M done")
                a2 = nc.vector.tensor_tensor(
                    out=M[:].rearrange("p j c -> p (j c)"),
                    in0=M[:].rearrange("p j c -> p (j c)"),
                    in1=bt[:].rearrange("p j c -> p (j c)"),
                    op=mybir.AluOpType.add,
                )
                add_dep_helper(a2.ins, bldA.ins, sync=True, reason="b loaded")
                a3 = nc.vector.tensor_scalar(
                    out=Xcat[:, :, 64 * (k + 1) : 64 * (k + 2)],
                    in0=M[:],
                    scalar1=0.0,
                    scalar2=None,
                    op0=mybir.AluOpType.max,
                )
                # 5) transpose agg into xT rows for next block (k<2)
                if k < 2:
                    tps = []
                    for t in range(JBLK):
                        pst = psp.tile([128, 128], f32, tag="tps")
                        tp = nc.tensor.transpose(
                            out=pst[0:64, :],
                            in_=Xcat[:, t, 64 * (k + 1) : 64 * (k + 2)],
                            identity=ident[:],
                        )
                        add_dep_helper(tp.ins, a3.ins, sync=True, reason="agg ready")
                        dst_tile = xT0 if k == 0 else xT1
                        ro = 64 if k == 0 else 0
                        cpt = nc.vector.tensor_copy(
                            out=dst_tile[ro : ro + 64, t * 128 : (t + 1) * 128],
                            in_=pst[0:64, :],
                        )
                        tps.append(cpt)
                    prev = tps[-1]
                else:
                    prev = a3

            # 6) final quad max-pool over Xcat channels (into M, which is free now)
            red = nc.vector.tensor_reduce(
                out=M[:].rearrange("p j c -> p (j c)"),
                in_=Xcat[:].rearrange("p j (g f) -> p (j g) f", f=4),
                op=mybir.AluOpType.max,
                axis=mybir.AxisListType.X,
            )
            add_dep_helper(red.ins, prev.ins, sync=True, reason="all blocks done")
            ow = nc.sync.dma_start(
                out=OutP[:].rearrange("(j p) c -> p j c", p=128),
                in_=M[:],
            )
            add_dep_helper(ow.ins, red.ins, sync=True, reason="out ready")

    nc.compile()

    # ---- host-side shard + run ----
    in_maps = []
    for m in range(N_CORES):
        perm = perms[m]
        xp = np.zeros((SLOTS, 64), np.float32)
        sel = perm >= 0
        xp[sel] = x[perm[sel]]
        xnm = np.zeros((128, JBLK, 64), np.float32)
        s = np.arange(SLOTS)
        xnm[s % 128, s // 128, :] = xp
        in_map = {
            "XT0in": np.ascontiguousarray(xp.T),
            "Xnm": xnm.reshape(128, -1),
            "IdxP": idx_all[m],
        }
        for k in range(3):
            in_map[f"W{k}"] = Ws[k]
            in_map[f"bb{k}"] = np.tile(bs[k][None, None, :], (128, JBLK, 1)).reshape(128, -1)
        in_maps.append(in_map)

    trace = os.environ.get("BASS_KERNEL_TRACE", "0") == "1" and _setup_trace_hook()
    res = run_bass_kernel_spmd(
        nc, in_maps, core_ids=list(range(N_CORES)), trace=trace
    )
    out = np.zeros((N_NODES, 64), np.float32)
    for m in range(N_CORES):
        o = res.results[m]["Out"]  # [SLOTS, 64] in slot order
        perm = perms[m]
        sel = perm >= 0
        out[perm[sel]] = o[sel]
    kernel._last_res = res
    return out


# revision 15
# speedup vs baseline: 1.0553x; 1.0553x over previous
"""DenseGCN (3x EdgeConv max-aggr) on 8 TRN2 NeuronCores.

Key algebra: EdgeConv message relu([x_d, x_s - x_d] @ W + b) with W=[Wt;Wb]
= relu(x_d@(Wt-Wb) + x_s@Wb + b). relu and the per-dst add are monotone, so
segment_max commutes: agg[v] = relu(A[v] + b + max_{e:dst=v} B[src_e]) with
A = x@Wt - x@Wb, B = x@Wb -- node-level matmuls only, no per-edge matmul.

Sharding: dst-partition nodes (2500/core, padded to 2560 slots, sorted by
in-degree desc). Per block: local matmuls (PE), AllGather of the B table
(f32 [8*2560+1, 64] with a -1e30 dummy row), then per-round 128-row
indirect-DMA gathers + DVE tensor max. Final quad max-pool on DVE.
"""
import sys, os, types

sys.path.insert(0, "/opt/trn_rl_repo")

import numpy as np


def _setup_trace_hook():
    """Register the NTFF profile hook (missing antenv.axon_hooks shim) so
    run_bass_kernel_spmd(trace=True) works. Safe no-op on failure."""
    try:
        import antenv

        if "antenv.axon_hooks" not in sys.modules:
            m = types.ModuleType("antenv.axon_hooks")
            hook = [None]
            m.set_axon_ntff_profile_hook = lambda h: hook.__setitem__(0, h)
            m.get_axon_ntff_profile_hook = lambda: hook[0]
            sys.modules["antenv.axon_hooks"] = m
            antenv.axon_hooks = m
            from trn_agent_boot.trn_boot import _ntff_profile_via_ctypes

            m.set_axon_ntff_profile_hook(
                _ntff_profile_via_ctypes("/opt/axon/libaxon_pjrt.so")
            )
        import concourse.bass_utils as bu

        bu.upload_artifacts = lambda tmpdir: tmpdir
        return True
    except Exception:
        return False

N_NODES = 20000
N_EDGES = 320000
C_IN = 64
GROWTH = 64
N_BLOCKS = 3
N_CORES = 8
NL = N_NODES // N_CORES          # 2500 local nodes
SLOTS = 2560                     # padded to 20*128
JBLK = SLOTS // 128              # 20 slot-blocks
TBL_ROWS = N_CORES * SLOTS + 1   # 20481, last row = -1e30 dummy
DUMMY = N_CORES * SLOTS


def _build_call_grid(edge_index):
    """Host-side (pure index manipulation): degree-sort nodes per core,
    build the per-round gather call grid, identical structure across cores."""
    src, dst = edge_index[0].astype(np.int64), edge_index[1].astype(np.int64)
    core_of = dst // NL
    perms = []          # per core: slot -> node (global id), len SLOTS (pad: -1)
    slot_of = np.full(N_NODES, -1, np.int64)
    edges_by_core = []
    for m in range(N_CORES):
        lo, hi = m * NL, (m + 1) * NL
        deg = np.bincount(dst[(dst >= lo) & (dst < hi)] - lo, minlength=NL)
        order = np.argsort(-deg, kind="stable")      # local node order by deg desc
        nodes = order + lo
        perm = np.concatenate([nodes, np.full(SLOTS - NL, -1, np.int64)])
        perms.append(perm)
        slot_of[nodes] = np.arange(NL)
        edges_by_core.append((deg, order))
    # global table row of node n: core(n)*SLOTS + local slot
    grow = (np.arange(N_NODES) // NL) * SLOTS + slot_of
    # per core, per slot: list of src rows (table row ids)
    per_core_srcs = []
    maxdeg_per_round = None
    for m in range(N_CORES):
        lo = m * NL
        mask = core_of == m
        d_loc = dst[mask] - lo
        s_rows = grow[src[mask]]
        sl = slot_of[dst[mask]]
        o = np.argsort(sl, kind="stable")
        sl, s_rows = sl[o], s_rows[o]
        starts = np.searchsorted(sl, np.arange(NL))
        ends = np.searchsorted(sl, np.arange(NL) + 1)
        per_core_srcs.append((starts, ends, s_rows))
    degs = np.stack([
        per_core_srcs[m][1] - per_core_srcs[m][0] for m in range(N_CORES)
    ])  # [8, NL] in slot order (desc within each core)
    R = int(degs.max())
    # n_r[r] = max over cores of count(slots with deg > r), rounded up to 128
    n_r = [SLOTS]  # round 0 covers all slots (direct write incl. dummy -1e30)
    for r in range(1, R):
        c = int((degs > r).sum(axis=1).max())
        n_r.append(((c + 127) // 128) * 128)
    # order calls by slot_block (deepest first): each slot block completes its
    # rounds contiguously so its epilogue/transpose/next-block matmul pipeline
    # under the remaining gather stream.
    calls = []  # list of (round, slot_block)
    for c in range(SLOTS // 128):
        for r, n in enumerate(n_r):
            if c < n // 128:
                calls.append((r, c))
    NCALLS = len(calls)
    idx_all = np.full((N_CORES, 128, NCALLS), DUMMY, np.int32)
    for m in range(N_CORES):
        starts, ends, s_rows = per_core_srcs[m]
        for k, (r, c) in enumerate(calls):
            base = c * 128
            for p in range(128):
                s = base + p
                if s < NL and starts[s] + r < ends[s]:
                    idx_all[m, p, k] = s_rows[starts[s] + r]
    return perms, calls, idx_all


def kernel(x, W0, b0, W1, b1, W2, b2, edge_index):
    import concourse.bacc as bacc
    import concourse.bass as bass
    import concourse.mybir as mybir
    import concourse.tile as tile
    from concourse.tile import add_dep_helper
    from concourse.masks import make_identity
    from concourse.bass_utils import run_bass_kernel_spmd

    x = np.asarray(x, np.float32)
    Ws = [np.asarray(W0, np.float32), np.asarray(W1, np.float32), np.asarray(W2, np.float32)]
    bs = [np.asarray(b0, np.float32), np.asarray(b1, np.float32), np.asarray(b2, np.float32)]
    edge_index = np.asarray(edge_index)

    perms, calls, idx_all = _build_call_grid(edge_index)
    NCALLS = len(calls)
    CS = [C_IN + k * GROWTH for k in range(N_BLOCKS)]   # 64,128,192
    WT_TILES = [1, 1, 2]  # number of 128-row sbuf tiles per Wt/Wb handled via slicing

    f32 = mybir.dt.float32
    nc = bacc.Bacc("TRN2", target_bir_lowering=False, debug=False, num_devices=N_CORES)

    XT0in = nc.declare_dram_parameter("XT0in", [64, SLOTS], f32, isOutput=False)
    Xnm = nc.declare_dram_parameter("Xnm", [128, JBLK * 64], f32, isOutput=False)
    Wp = [nc.declare_dram_parameter(f"W{k}", [2 * CS[k], 64], f32, isOutput=False) for k in range(3)]
    Bb = [nc.declare_dram_parameter(f"bb{k}", [128, JBLK * 64], f32, isOutput=False) for k in range(3)]
    IdxP = nc.declare_dram_parameter("IdxP", [128, NCALLS], mybir.dt.int32, isOutput=False)
    OutP = nc.declare_dram_parameter("Out", [SLOTS, 64], f32, isOutput=True)

    Btab = nc.dram_tensor("btab", [TBL_ROWS, 64], f32)
    Bloc = nc.dram_tensor("bloc", [SLOTS, 64], f32)

    with tile.TileContext(nc) as tc:
        with (
            tc.tile_pool(name="big", bufs=1) as bigp,
            tc.tile_pool(name="g", bufs=8) as gp,
            tc.tile_pool(name="ps", bufs=2, space="PSUM") as psp,
        ):
            xT0 = bigp.tile([128, SLOTS], f32, tag="xT0")
            xT1 = bigp.tile([128, SLOTS], f32, tag="xT1")
            Xcat = bigp.tile([128, JBLK, 256], f32, tag="Xcat")
            M = bigp.tile([128, JBLK, 64], f32, tag="M")
            As = bigp.tile([128, JBLK, 64], f32, tag="As")
            Bs = bigp.tile([128, JBLK, 64], f32, tag="Bs")
            Idx = bigp.tile([128, NCALLS], mybir.dt.int32, tag="Idx")
            # W chunk layout: each (block, part, chunk) gets its own 64-col slot, rows [0:K)
            wchunks = {}  # (k, part) -> list of (slot, row_in_W, K, x_chan_offset)
            slot = 0
            for k in range(N_BLOCKS):
                C = CS[k]
                for part in (0, 1):
                    lst = []
                    r = 0
                    while r < C:
                        kk = min(128, C - r)
                        lst.append((slot, part * C + r, kk, r))
                        slot += 1
                        r += kk
                    wchunks[(k, part)] = lst
            NSLOTS = slot
            Wt = bigp.tile([128, NSLOTS * 64], f32, tag="Wt")
            bt = bigp.tile([128, JBLK, 64], f32, tag="bt")
            ident = bigp.tile([128, 128], f32, tag="ident")
            neg = bigp.tile([128, 64], f32, tag="neg")

            make_identity(nc, ident[:])
            ld = []
            ld.append(nc.sync.dma_start(out=xT0[0:64, :], in_=XT0in[:]))
            ld.append(nc.sync.dma_start(out=Xcat[:, :, 0:64], in_=Xnm[:].rearrange("p (j c) -> p j c", c=64)))
            ld.append(nc.sync.dma_start(out=Idx[:], in_=IdxP[:]))
            for k in range(3):
                for part in (0, 1):
                    for (sl, wrow, kk, xoff) in wchunks[(k, part)]:
                        ld.append(nc.sync.dma_start(
                            out=Wt[0:kk, sl * 64 : (sl + 1) * 64],
                            in_=Wp[k][wrow : wrow + kk, :],
                        ))
            mneg = nc.gpsimd.memset(neg[:], -1e30)
            dummy_w = nc.sync.dma_start(out=Btab[DUMMY : DUMMY + 1, :], in_=neg[0:1, :])
            add_dep_helper(dummy_w.ins, mneg.ins, sync=True, reason="dummy row after memset")

            # per-slot-block call lists: c -> [(kcall, r), ...] in round order
            calls_by_c = {}
            for kcall, (r, c) in enumerate(calls):
                calls_by_c.setdefault(c, []).append((kcall, r))

            def matmul_into(k, part, t, tag):
                """x_k @ (Wt|Wb) for node tile t -> PSUM tile (returned)."""
                ps = psp.tile([128, 64], f32, tag=tag)
                chunks = wchunks[(k, part)]
                for ci, (sl, wrow, kk, xoff) in enumerate(chunks):
                    xt_tile = xT0 if xoff < 128 else xT1
                    xo = xoff if xoff < 128 else xoff - 128
                    nc.tensor.matmul(
                        out=ps[:, :],
                        lhsT=xt_tile[xo : xo + kk, t * 128 : (t + 1) * 128],
                        rhs=Wt[0:kk, sl * 64 : (sl + 1) * 64],
                        start=(ci == 0),
                        stop=(ci == len(chunks) - 1),
                    )
                return ps

            prev_block_done = [dummy_w]  # insts that must precede AllGather (WAR)
            prev = dummy_w
            for k in range(N_BLOCKS):
                C = CS[k]
                bldA = nc.sync.dma_start(out=bt[:].rearrange("p j c -> p (j c)"), in_=Bb[k][:])

                # 1) B-part matmuls first (they gate the collective); write per-t
                #    slabs straight to DRAM so the AllGather can start asap.
                bws = []
                for t in range(JBLK):
                    ps = matmul_into(k, 1, t, "mmB")
                    cp = nc.vector.tensor_copy(out=Bs[:, t, :], in_=ps[:, :])
                    bw = nc.sync.dma_start(
                        out=Bloc[t * 128 : (t + 1) * 128, :], in_=Bs[:, t, :]
                    )
                    add_dep_helper(bw.ins, cp.ins, sync=True, reason="B_t ready")
                    bws.append(bw)
                ag = nc.gpsimd.collective_compute(
                    "AllGather", mybir.AluOpType.bypass,
                    replica_groups=[list(range(N_CORES))],
                    ins=[Bloc[:]],
                    outs=[Btab[0 : N_CORES * SLOTS, :]],
                )
                for bw in bws:
                    add_dep_helper(ag.ins, bw.ins, sync=True, reason="allgather after write")
                for d in prev_block_done:
                    add_dep_helper(ag.ins, d.ins, sync=True, reason="WAR: prior block gathers done")

                # 2) A-part matmuls (P) -> As; A = P - B. Not on the collective's
                #    critical path; they fill PE/DVE time under the gather stream.
                subs = {}
                for t in range(JBLK):
                    ps = matmul_into(k, 0, t, "mmA")
                    cp = nc.vector.tensor_copy(out=As[:, t, :], in_=ps[:, :])
                    sub = nc.vector.tensor_tensor(
                        out=As[:, t, :], in0=As[:, t, :], in1=Bs[:, t, :],
                        op=mybir.AluOpType.subtract,
                    )
                    subs[t] = sub

                # 3) per slot-block: gather rounds, then epilogue + transpose,
                #    pipelined under the remaining slot-blocks' gather stream.
                block_done = []
                last_a3 = None
                for c in range(JBLK):
                    last_mx = None
                    for kcall, r in calls_by_c[c]:
                        if r == 0:
                            gcall = nc.gpsimd.indirect_dma_start(
                                out=M[:, c, :], out_offset=None, in_=Btab[:],
                                in_offset=bass.IndirectOffsetOnAxis(ap=Idx[:, kcall : kcall + 1], axis=0),
                            )
                            add_dep_helper(gcall.ins, ag.ins, sync=True, reason="table ready")
                            last_mx = gcall
                        else:
                            gt = gp.tile([128, 64], f32, tag="gt")
                            gcall = nc.gpsimd.indirect_dma_start(
                                out=gt[:, :], out_offset=None, in_=Btab[:],
                                in_offset=bass.IndirectOffsetOnAxis(ap=Idx[:, kcall : kcall + 1], axis=0),
                            )
                            add_dep_helper(gcall.ins, ag.ins, sync=True, reason="table ready")
                            mx = nc.vector.tensor_tensor(
                                out=M[:, c, :], in0=M[:, c, :], in1=gt[:, :],
                                op=mybir.AluOpType.max,
                            )
                            add_dep_helper(mx.ins, gcall.ins, sync=True, reason="gather done")
                            last_mx = mx
                    # epilogue for this slot block: agg = relu(A + b + M)
                    e1 = nc.vector.tensor_tensor(
                        out=M[:, c, :], in0=M[:, c, :], in1=As[:, c, :],
                        op=mybir.AluOpType.add,
                    )
                    add_dep_helper(e1.ins, last_mx.ins, sync=True, reason="M_c done")
                    add_dep_helper(e1.ins, subs[c].ins, sync=True, reason="A_c ready")
                    e2 = nc.vector.tensor_tensor(
                        out=M[:, c, :], in0=M[:, c, :], in1=bt[:, c, :],
                        op=mybir.AluOpType.add,
                    )
                    add_dep_helper(e2.ins, bldA.ins, sync=True, reason="b loaded")
                    e3 = nc.vector.tensor_scalar(
                        out=Xcat[:, c, 64 * (k + 1) : 64 * (k + 2)],
                        in0=M[:, c, :],
                        scalar1=0.0,
                        scalar2=None,
                        op0=mybir.AluOpType.max,
                    )
                    last_a3 = e3
                    block_done.append(e3)
                    if k < 2:
                        pst = psp.tile([128, 128], f32, tag="tps")
                        tp = nc.tensor.transpose(
                            out=pst[0:64, :],
                            in_=Xcat[:, c, 64 * (k + 1) : 64 * (k + 2)],
                            identity=ident[:],
                        )
                        add_dep_helper(tp.ins, e3.ins, sync=True, reason="agg_c ready")
                        dst_tile = xT0 if k == 0 else xT1
                        ro = 64 if k == 0 else 0
                        cpt = nc.vector.tensor_copy(
                            out=dst_tile[ro : ro + 64, c * 128 : (c + 1) * 128],
                            in_=pst[0:64, :],
                        )
                        block_done.append(cpt)
                prev_block_done = block_done
                prev = last_a3

            # 6) final quad max-pool over Xcat channels (into M, which is free now)
            red = nc.vector.tensor_reduce(
                out=M[:].rearrange("p j c -> p (j c)"),
                in_=Xcat[:].rearrange("p j (g f) -> p (j g) f", f=4),
                op=mybir.AluOpType.max,
                axis=mybir.AxisListType.X,
            )
            add_dep_helper(red.ins, prev.ins, sync=True, reason="all blocks done")
            ow = nc.sync.dma_start(
                out=OutP[:].rearrange("(j p) c -> p j c", p=128),
                in_=M[:],
            )
            add_dep_helper(ow.ins, red.ins, sync=True, reason="out ready")

    nc.compile()

    # ---- host-side shard + run ----
    in_maps = []
    for m in range(N_CORES):
        perm = perms[m]
        xp = np.zeros((SLOTS, 64), np.float32)
        sel = perm >= 0
        xp[sel] = x[perm[sel]]
        xnm = np.zeros((128, JBLK, 64), np.float32)
        s = np.arange(SLOTS)
        xnm[s % 128, s // 128, :] = xp
        in_map = {
            "XT0in": np.ascontiguousarray(xp.T),
            "Xnm": xnm.reshape(128, -1),
            "IdxP": idx_all[m],
        }
        for k in range(3):
            in_map[f"W{k}"] = Ws[k]
            in_map[f"bb{k}"] = np.tile(bs[k][None, None, :], (128, JBLK, 1)).reshape(128, -1)
        in_maps.append(in_map)

    trace = os.environ.get("BASS_KERNEL_TRACE", "0") == "1" and _setup_trace_hook()
    res = run_bass_kernel_spmd(
        nc, in_maps, core_ids=list(range(N_CORES)), trace=trace
    )
    out = np.zeros((N_NODES, 64), np.float32)
    for m in range(N_CORES):
        o = res.results[m]["Out"]  # [SLOTS, 64] in slot order
        perm = perms[m]
        sel = perm >= 0
        out[perm[sel]] = o[sel]
    kernel._last_res = res
    return out
